# revision 23
# baseline (speedup 1.0000x reference)
"""Trainium2 Bass kernel for the e3nn-style InterModule:
   out = Linear2( NormAct( Linear1(x) ) )  over irreps
     IN  [(512,0),(256,1),(128,2)]  dim 1920
     MID [(1024,0),(512,1),(256,2)] dim 3840
     OUT = IN

v2 strategy (data-parallel over N across 8 cores, 8 blocks of 512 nodes/core):
  - Host: x column-permuted to j-major layout ([l0 u | l1 j,u | l2 j,u])
    and cast to bf16; weights prescaled by 1/sqrt(mul_in), bf16.
  - xt loaded feature-major straight from DRAM via XBAR dma transpose
    (2-byte path) -> no PE transposes at all.
  - Linear1: stationary W1 (bf16), moving xt -> h in PSUM (f32).
  - NormAct: phase A (sqrt ACT table): ACT copies h->g (bf16), DVE squares
    (2x), Pool j-sum adds, ACT sqrt; phase B (sigmoid table): DVE abs (4x),
    ACT sigmoids, DVE scales (2x) in place on g.
  - Linear2: stationary g chunks (bf16), moving W2 -> node-major out in
    PSUM; copies to bf16 outsb (j-major cols); DMA out; host inverse-
    permutes columns and upcasts to f32. L2 runs one block behind L1 so
    PE never waits on the normact chain.
"""

import math
from contextlib import ExitStack

import numpy as np
import ml_dtypes

import concourse.bass as bass
import concourse.tile as tile
from concourse import bacc, mybir
from concourse.bass_utils import run_bass_kernel_spmd

F32 = mybir.dt.float32
BF16 = mybir.dt.bfloat16
AF = mybir.ActivationFunctionType
ALU = mybir.AluOpType

N_CORES = 8
N_TOTAL = 32768
N_CORE = N_TOTAL // N_CORES          # 4096
BLK = 512                            # nodes per block
NBLK = N_CORE // BLK                 # 8
NSUB = BLK // 128                    # 4

D_IN = 1920
D_OUT = 1920

# permuted feature-tile order (128 features each):
#   ft 0..3   : l0, u-chunks
#   ft 4+j*2+k: l1, j in 0..2, k (u-chunk) in 0..1
#   ft 10+j   : l2, j in 0..4
N_FT = 15


def _col_perm_in():
    """orig col -> position such that x_p = x[:, perm] is j-major."""
    perm = np.empty(D_IN, dtype=np.int64)
    pos = 0
    for u in range(512):
        perm[pos] = u
        pos += 1
    for j in range(3):
        for u in range(256):
            perm[pos] = 512 + u * 3 + j
            pos += 1
    for j in range(5):
        for u in range(128):
            perm[pos] = 1280 + u * 5 + j
            pos += 1
    return perm


def _col_gather_out():
    """index g such that out_full = out_p[:, g] restores reference order."""
    g = np.empty(D_OUT, dtype=np.int64)
    for v in range(512):
        g[v] = v
    for v in range(256):
        for j in range(3):
            g[512 + v * 3 + j] = 512 + j * 256 + v
    for v in range(128):
        for j in range(5):
            g[1280 + v * 5 + j] = 1280 + j * 128 + v
    return g


_PERM_IN = _col_perm_in()
_GATHER_OUT = _col_gather_out()


def _build():
    nc = bacc.Bacc(
        "TRN2", target_bir_lowering=False, debug=False, num_devices=N_CORES
    )

    x = nc.dram_tensor("x", [N_CORE, D_IN], BF16, kind="ExternalInput").ap()
    w1_l0 = nc.dram_tensor("w1_l0", [512, 1024], BF16, kind="ExternalInput").ap()
    w1_l1 = nc.dram_tensor("w1_l1", [256, 512], BF16, kind="ExternalInput").ap()
    w1_l2 = nc.dram_tensor("w1_l2", [128, 256], BF16, kind="ExternalInput").ap()
    w2_l0 = nc.dram_tensor("w2_l0", [1024, 512], BF16, kind="ExternalInput").ap()
    w2_l1 = nc.dram_tensor("w2_l1", [512, 256], BF16, kind="ExternalInput").ap()
    w2_l2 = nc.dram_tensor("w2_l2", [256, 128], BF16, kind="ExternalInput").ap()
    out = nc.dram_tensor("out", [N_CORE, D_OUT], BF16, kind="ExternalOutput").ap()

    with tile.TileContext(nc) as tc, ExitStack() as ctx:
        consts = ctx.enter_context(tc.tile_pool(name="consts", bufs=1))
        sb = ctx.enter_context(tc.tile_pool(name="sb", bufs=1))
        ps = ctx.enter_context(tc.tile_pool(name="ps", bufs=1, space="PSUM"))

        # ---- weights: straight DMA into bf16 tiles ----
        w1r_l0 = consts.tile([128, 4, 1024], BF16)
        w1r_l1 = consts.tile([128, 2, 512], BF16)
        w1r_l2 = consts.tile([128, 256], BF16)
        w2r_l0 = consts.tile([128, 8, 512], BF16)
        w2r_l1 = consts.tile([128, 4, 256], BF16)
        w2r_l2 = consts.tile([128, 2, 128], BF16)
        xts = [None] * NBLK

        def load_xt(b):
            xt = sb.tile([128, N_FT, BLK], BF16, name=f"xt{b % 3}", tag="xt", bufs=3)
            # one XBAR transpose per block: the middle dim (feature tiles)
            # extends the partition dim, i.e. xt[p, m, n] = x[n, m*128+p]
            nc.sync.dma_start_transpose(
                out=xt, in_=x[b * BLK : (b + 1) * BLK, :]
            )
            xts[b] = xt

        def phase_a(b, state, interleave=None):
            """Emit L1 + the full normact for block b, with the l1/l2 norm
            chain (sqrt -> sigmoid -> scales) hoisted BEFORE the l0 MM
            section so the scales complete inside this block's PE window.
            interleave() emits one L2(b-1) ns-chunk at chosen boundaries so
            every engine queue serves L2(b-1) promptly while PE runs it."""
            xt = xts[b]
            g_l0 = sb.tile([128, 8, BLK], BF16, name="g_l0", tag="g_l0", bufs=2)
            g_l1 = sb.tile([128, 4, 3, BLK], BF16, name="g_l1", tag="g_l1", bufs=2)
            g_l2 = sb.tile([128, 2, 5, BLK], BF16, name="g_l2", tag="g_l2", bufs=2)
            nbuf = sb.tile([128, 6, BLK], BF16, name="nbuf", tag="nbuf", bufs=2)
            s6 = sb.tile([128, 6, BLK], BF16, name="s6", tag="s6", bufs=1)
            a = sb.tile([128, 8, BLK], BF16, name="a_l0", tag="a_l0", bufs=1)

            def il():
                if interleave is not None:
                    interleave()

            # l1: mid muls 512 -> kv 0..3; contraction 256 -> ki 0..1
            for kv in range(4):
                h = ps.tile([128, 3, BLK], F32, name="h1", tag="ps3", bufs=2)
                for j in range(3):
                    for ki in range(2):
                        nc.tensor.matmul(
                            h[:, j, :],
                            w1r_l1[:, ki, kv * 128 : (kv + 1) * 128],
                            xt[:, 4 + j * 2 + ki, :],
                            start=(ki == 0),
                            stop=(ki == 1),
                        )
                nc.scalar.activation(out=g_l1[:, kv], in_=h, func=AF.Copy)
                sq = sb.tile([128, 3, BLK], BF16, name="sq3", tag="sq1", bufs=3)
                nc.vector.tensor_mul(sq, g_l1[:, kv], g_l1[:, kv])
                nc.gpsimd.tensor_add(nbuf[:, kv, :], sq[:, 0, :], sq[:, 1, :])
                nc.gpsimd.tensor_add(nbuf[:, kv, :], nbuf[:, kv, :], sq[:, 2, :])
                if kv == 1 or kv == 3:
                    il()
            # l2: mid muls 256 -> kv 0..1; contraction 128 single chunk
            for kv in range(2):
                hA = ps.tile([128, 3, BLK], F32, name="h2a", tag="ps3", bufs=2)
                for j in range(3):
                    nc.tensor.matmul(
                        hA[:, j, :],
                        w1r_l2[:, kv * 128 : (kv + 1) * 128],
                        xt[:, 10 + j, :],
                        start=True,
                        stop=True,
                    )
                hB = ps.tile([128, 2, BLK], F32, name="h2b", tag="ps3", bufs=2)
                for j in range(3, 5):
                    nc.tensor.matmul(
                        hB[:, j - 3, :],
                        w1r_l2[:, kv * 128 : (kv + 1) * 128],
                        xt[:, 10 + j, :],
                        start=True,
                        stop=True,
                    )
                nc.scalar.activation(out=g_l2[:, kv, 0:3], in_=hA, func=AF.Copy)
                nc.scalar.activation(out=g_l2[:, kv, 3:5], in_=hB, func=AF.Copy)
                sq5 = sb.tile([128, 5, BLK], BF16, name="sq5", tag="sq2", bufs=2)
                nc.vector.tensor_mul(sq5, g_l2[:, kv], g_l2[:, kv])
                nsq = nbuf[:, 4 + kv, :]
                nc.gpsimd.tensor_add(nsq, sq5[:, 0, :], sq5[:, 1, :])
                nc.gpsimd.tensor_add(nsq, nsq, sq5[:, 2, :])
                nc.gpsimd.tensor_add(nsq, nsq, sq5[:, 3, :])
                nc.gpsimd.tensor_add(nsq, nsq, sq5[:, 4, :])
            il()
            U16 = mybir.dt.uint16

            def l0_finale(lo, hi):
                # s = sigmoid(|h|); |x| on bf16 = clear the sign bit.
                # Sigmoid stays in the already-loaded sigmoid table set.
                nc.vector.tensor_scalar(
                    out=a[:, lo:hi].bitcast(U16),
                    in0=g_l0[:, lo:hi].bitcast(U16),
                    scalar1=0x7FFF,
                    scalar2=None,
                    op0=ALU.bitwise_and,
                )
                nc.scalar.activation(out=a[:, lo:hi], in_=a[:, lo:hi], func=AF.Sigmoid)
                nc.vector.tensor_mul(g_l0[:, lo:hi], g_l0[:, lo:hi], a[:, lo:hi])

            # l0: mid muls 1024 -> kv 0..7 in triples; contraction 512 -> ki 0..3.
            # The norm chains are spread between the l0 copies so ACT's block
            # tail stays short and the ring-releasing copies are prompt.
            for t, kvs in enumerate(((0, 1, 2), (3, 4, 5), (6, 7))):
                h = ps.tile([128, len(kvs), BLK], F32, name=f"h0_{t}", tag="ps3", bufs=2)
                for i, kv in enumerate(kvs):
                    for ki in range(4):
                        nc.tensor.matmul(
                            h[:, i, :],
                            w1r_l0[:, ki, kv * 128 : (kv + 1) * 128],
                            xt[:, ki, :],
                            start=(ki == 0),
                            stop=(ki == 3),
                        )
                nc.scalar.activation(
                    out=g_l0[:, kvs[0] : kvs[0] + len(kvs)], in_=h, func=AF.Copy
                )
                if t == 0:
                    il()
                    # l1/l2 norm chain: sqrt (sqrt table), sigmoid (sigmoid
                    # table), then DVE scales
                    nc.scalar.activation(out=nbuf, in_=nbuf, func=AF.Sqrt)
                    nc.scalar.activation(out=s6, in_=nbuf, func=AF.Sigmoid)
                    for kv in range(4):
                        nc.vector.tensor_mul(
                            g_l1[:, kv],
                            g_l1[:, kv],
                            s6[:, kv, :].unsqueeze(1).broadcast_to([128, 3, BLK]),
                        )
                    for kv in range(2):
                        nc.vector.tensor_mul(
                            g_l2[:, kv],
                            g_l2[:, kv],
                            s6[:, 4 + kv, :].unsqueeze(1).broadcast_to([128, 5, BLK]),
                        )
                elif t == 1:
                    il()
                    l0_finale(0, 6)
            l0_finale(6, 8)
            il()
            state.update(g_l0=g_l0, g_l1=g_l1, g_l2=g_l2)

        def linear2_ns(b, state, ns):
            g_l0, g_l1, g_l2 = state["g_l0"], state["g_l1"], state["g_l2"]
            nsl = slice(ns * 128, (ns + 1) * 128)
            outsb = sb.tile([128, D_OUT], BF16, name="outsb", tag="outsb", bufs=3)
            # l1: out muls 256, contraction 512 -> ku 0..3 (j-major out cols)
            q1 = ps.tile([128, 2, 256], F32, name="q1", tag="ps1", bufs=2)
            for jj, j in enumerate((0, 1)):
                for ku in range(4):
                    nc.tensor.matmul(
                        q1[:, jj, :],
                        g_l1[:, ku, j, nsl],
                        w2r_l1[:, ku, :],
                        start=(ku == 0),
                        stop=(ku == 3),
                    )
            nc.vector.tensor_copy(out=outsb[:, 512:1024], in_=q1)
            q2 = ps.tile([128, 256], F32, name="q2", tag="ps1", bufs=2)
            for ku in range(4):
                nc.tensor.matmul(
                    q2,
                    g_l1[:, ku, 2, nsl],
                    w2r_l1[:, ku, :],
                    start=(ku == 0),
                    stop=(ku == 3),
                )
            nc.vector.tensor_copy(out=outsb[:, 1024:1280], in_=q2)
            # l2: out muls 128, contraction 256 -> ku 0..1
            q3 = ps.tile([128, 4, 128], F32, name="q3", tag="ps1", bufs=2)
            for j in range(4):
                for ku in range(2):
                    nc.tensor.matmul(
                        q3[:, j, :],
                        g_l2[:, ku, j, nsl],
                        w2r_l2[:, ku, :],
                        start=(ku == 0),
                        stop=(ku == 1),
                    )
            nc.vector.tensor_copy(out=outsb[:, 1280:1792], in_=q3)
            q4 = ps.tile([128, 128], F32, name="q4", tag="ps1", bufs=2)
            for ku in range(2):
                nc.tensor.matmul(
                    q4,
                    g_l2[:, ku, 4, nsl],
                    w2r_l2[:, ku, :],
                    start=(ku == 0),
                    stop=(ku == 1),
                )
            nc.vector.tensor_copy(out=outsb[:, 1792:1920], in_=q4)
            # l0 last: its scale chain is the last to finish in phase A
            q0 = ps.tile([128, 512], F32, name="q0", tag="ps1", bufs=2)
            for ku in range(8):
                nc.tensor.matmul(
                    q0,
                    g_l0[:, ku, nsl],
                    w2r_l0[:, ku, :],
                    start=(ku == 0),
                    stop=(ku == 7),
                )
            nc.vector.tensor_copy(out=outsb[:, 0:512], in_=q0)
            nc.sync.dma_start(
                out=out[b * BLK + ns * 128 : b * BLK + (ns + 1) * 128, :],
                in_=outsb,
            )

        # ---- software-pipelined main loop: L2 one block behind L1, with
        # its per-ns chunks emission-interleaved into phase A so every
        # engine queue serves L2(b-1) promptly while PE executes it ----
        # prologue ordered so block 0's l1 matmuls can start ASAP:
        # w1_l1, then the l1 feature tiles of xt(0), then everything else
        nc.sync.dma_start(out=w1r_l1, in_=w1_l1.rearrange("(t p) v -> p t v", p=128))
        xt0 = sb.tile([128, N_FT, BLK], BF16, name="xt0", tag="xt", bufs=3)
        nc.sync.dma_start_transpose(out=xt0[:, 4:10, :], in_=x[0:BLK, 512:1280])
        nc.sync.dma_start(out=w1r_l2, in_=w1_l2)
        nc.sync.dma_start_transpose(out=xt0[:, 10:15, :], in_=x[0:BLK, 1280:1920])
        nc.sync.dma_start(out=w1r_l0, in_=w1_l0.rearrange("(t p) v -> p t v", p=128))
        nc.sync.dma_start_transpose(out=xt0[:, 0:4, :], in_=x[0:BLK, 0:512])
        xts[0] = xt0
        nc.sync.dma_start(out=w2r_l1, in_=w2_l1.rearrange("(t p) v -> p t v", p=128))
        nc.sync.dma_start(out=w2r_l2, in_=w2_l2.rearrange("(t p) v -> p t v", p=128))
        nc.sync.dma_start(out=w2r_l0, in_=w2_l0.rearrange("(t p) v -> p t v", p=128))
        load_xt(1)
        states = [dict() for _ in range(NBLK)]

        def l2_stepper(b):
            counter = [0]

            def step():
                if counter[0] < NSUB:
                    linear2_ns(b, states[b], counter[0])
                    counter[0] += 1

            return step

        for b in range(NBLK):
            if b + 2 < NBLK:
                load_xt(b + 2)
            il = l2_stepper(b - 1) if b > 0 else None
            phase_a(b, states[b], interleave=il)
            if il is not None:
                for _ in range(NSUB):
                    il()
        fin = l2_stepper(NBLK - 1)
        for _ in range(NSUB):
            fin()

    nc.compile()
    return nc


_NC_CACHE = None


def _get_nc():
    global _NC_CACHE
    if _NC_CACHE is None:
        _NC_CACHE = _build()
    return _NC_CACHE


def _prepare(x, w1_l0, w1_l1, w1_l2, w2_l0, w2_l1, w2_l2):
    """Host-side prep: column-permute + bf16-cast x, prescale + bf16 weights.
    Returns per-core input maps."""
    bf = ml_dtypes.bfloat16
    x_p = np.ascontiguousarray(
        np.asarray(x, np.float32)[:, _PERM_IN].astype(bf)
    )
    ws = {
        "w1_l0": np.asarray(w1_l0, np.float32) / math.sqrt(512.0),
        "w1_l1": np.asarray(w1_l1, np.float32) / math.sqrt(256.0),
        "w1_l2": np.asarray(w1_l2, np.float32) / math.sqrt(128.0),
        "w2_l0": np.asarray(w2_l0, np.float32) / math.sqrt(1024.0),
        "w2_l1": np.asarray(w2_l1, np.float32) / math.sqrt(512.0),
        "w2_l2": np.asarray(w2_l2, np.float32) / math.sqrt(256.0),
    }
    ws = {k: np.ascontiguousarray(v.astype(bf)) for k, v in ws.items()}
    return [
        {"x": x_p[c * N_CORE : (c + 1) * N_CORE], **ws} for c in range(N_CORES)
    ]


def _gather(res):
    out_p = np.concatenate([res[c]["out"] for c in range(N_CORES)], axis=0)
    return np.ascontiguousarray(out_p[:, _GATHER_OUT].astype(np.float32))


def kernel(x, w1_l0, w1_l1, w1_l2, w2_l0, w2_l1, w2_l2):
    in_maps = _prepare(x, w1_l0, w1_l1, w1_l2, w2_l0, w2_l1, w2_l2)
    nc = _get_nc()
    res = run_bass_kernel_spmd(nc, in_maps, list(range(N_CORES))).results
    return _gather(res)
